# revision 4
# baseline (speedup 1.0000x reference)
"""BSFSNet (topk_masking) Trainium2 kernel.

Pure data-parallel over 8 NeuronCores: batch B=1024 split into 8 shards of
128 rows; selector/backbone weights replicated.

Per-core pipeline:
  1. S = x @ W_s + b_s            (PE, fp32, PSUM-accumulated over 8 K-chunks)
  2. per (row, head): exact top-k thresholds for k in {32,64,128,256} via
     iterative 8-at-a-time extraction (vector-engine max + match_replace).
     The 8th value of extraction blocks 4/8/16/32 is exactly the k-th
     largest (matches jax.lax.top_k semantics including duplicates).
  3. masks M = sigmoid((S - kth)/tau)  (scalar engine, per-partition bias)
  4. x_masked = x * M  (gpsimd), transposed on PE for the backbone matmuls
  5. h^T = relu(W1^T @ xm^T + b1); logits^T = W2^T @ h^T + b2  (PE + ACT)
  6. Y written back transposed; softmax over classes of the k=256 slice,
     mean over heads -> final_probs.
"""

import sys

if "/opt/trn_rl_repo" not in sys.path:
    sys.path.insert(0, "/opt/trn_rl_repo")

import numpy as np

B, F, H, C = 1024, 1024, 128, 100
KFC = 8                      # ranker heads
KLIST = (32, 64, 128, 256)   # hierarchical subset sizes
KSB = len(KLIST)
NCORES = 8
BS = B // NCORES             # batch rows per core
NEG = -3.0e38                # replacement value for extracted maxima

_CACHE = {}
_TRACE = False        # set by test harness to capture an NTFF profile
_LAST_RES = None      # last BassKernelResults (exec_time_ns etc.)


def _build(inv_tau: float):
    from concourse import bacc, mybir
    from concourse import tile
    from concourse.masks import make_identity

    f32 = mybir.dt.float32
    nc = bacc.Bacc("TRN2", target_bir_lowering=False, debug=False)

    x_d = nc.declare_dram_parameter("x", [BS, F], f32, isOutput=False)
    ws_d = nc.declare_dram_parameter("W_s", [F, KFC * F], f32, isOutput=False)
    bs_d = nc.declare_dram_parameter("b_s", [1, KFC * F], f32, isOutput=False)
    w1_d = nc.declare_dram_parameter("W1", [F, H], f32, isOutput=False)
    b1_d = nc.declare_dram_parameter("b1", [H, 1], f32, isOutput=False)
    w2_d = nc.declare_dram_parameter("W2", [H, C], f32, isOutput=False)
    b2_d = nc.declare_dram_parameter("b2", [C, 1], f32, isOutput=False)

    probs_d = nc.declare_dram_parameter("probs", [BS, C], f32, isOutput=True)
    y_d = nc.declare_dram_parameter("Y", [BS, KFC, KSB, C], f32, isOutput=True)
    m_d = nc.declare_dram_parameter("M", [BS, KFC, KSB, F], f32, isOutput=True)
    s_d = nc.declare_dram_parameter("S", [BS, KFC * F], f32, isOutput=True)

    AF = mybir.ActivationFunctionType
    AX = mybir.AxisListType

    with tile.TileContext(nc) as tc:
        with (
            tc.tile_pool(name="const", bufs=1) as const,
            tc.tile_pool(name="wstream", bufs=6) as wpool,
            tc.tile_pool(name="scr", bufs=4) as spool,
            tc.tile_pool(name="th", bufs=8) as thpool,
            tc.tile_pool(name="mask", bufs=4) as mpool,
            tc.tile_pool(name="xm", bufs=3) as xmpool,
            tc.tile_pool(name="xmt", bufs=3) as xtpool,
            tc.tile_pool(name="bb", bufs=4) as bbpool,
            tc.tile_pool(name="tiny", bufs=16) as tiny,
            tc.tile_pool(name="psS", bufs=2, space="PSUM") as psS,
            tc.tile_pool(name="psT", bufs=2, space="PSUM") as psT,
            tc.tile_pool(name="psH", bufs=2, space="PSUM") as psH,
            tc.tile_pool(name="psL", bufs=1, space="PSUM") as psL,
        ):
            identity = const.tile([128, 128], f32)
            make_identity(nc, identity)
            ones1 = const.tile([1, 128], f32)
            nc.gpsimd.memset(ones1, 1.0)

            xsb = const.tile([BS, F], f32)
            nc.sync.dma_start(out=xsb, in_=x_d[:, :])
            bs_sb = const.tile([1, KFC * F], f32)
            nc.sync.dma_start(out=bs_sb, in_=bs_d[:, :])
            w1t = const.tile([128, 8, H], f32)
            for fc in range(8):
                nc.sync.dma_start(out=w1t[:, fc, :], in_=w1_d[fc * 128:(fc + 1) * 128, :])
            w2sb = const.tile([H, C], f32)
            nc.sync.dma_start(out=w2sb, in_=w2_d[:, :])
            b1sb = const.tile([H, 1], f32)
            nc.sync.dma_start(out=b1sb, in_=b1_d[:, :])
            b2sb = const.tile([C, 1], f32)
            nc.sync.dma_start(out=b2sb, in_=b2_d[:, :])

            # x^T tiles for the selector matmul
            xT = const.tile([128, 8, BS], f32)
            for fc in range(8):
                pt = psT.tile([128, 128], f32)
                nc.tensor.transpose(pt, xsb[:, fc * 128:(fc + 1) * 128], identity)
                nc.scalar.copy(xT[:, fc, :], pt)

            # ---- selector: S = x @ W_s + b_s, [BS, 8192] resident in SBUF
            S_sb = const.tile([BS, KFC * F], f32)
            for sc in range(16):
                ps = psS.tile([128, 512], f32)
                for fc in range(8):
                    wst = wpool.tile([128, 512], f32)
                    nc.sync.dma_start(
                        out=wst,
                        in_=ws_d[fc * 128:(fc + 1) * 128, sc * 512:(sc + 1) * 512],
                    )
                    nc.tensor.matmul(ps, xT[:, fc, :], wst, start=(fc == 0), stop=False)
                # += broadcast(b_s) via K=1 matmul of ones^T @ b_s-slice
                nc.tensor.matmul(
                    ps, ones1, bs_sb[0:1, sc * 512:(sc + 1) * 512],
                    start=False, stop=True,
                )
                nc.scalar.copy(S_sb[:, sc * 512:(sc + 1) * 512], ps)
                nc.sync.dma_start(
                    out=s_d[:, sc * 512:(sc + 1) * 512],
                    in_=S_sb[:, sc * 512:(sc + 1) * 512],
                )

            # ---- per head: extract top-256 8-at-a-time; thresholds at 32/64/128/256
            kk_of_iter = {4: 0, 8: 1, 16: 2, 32: 3}
            nth_all = []
            for h in range(KFC):
                s_head = S_sb[:, h * F:(h + 1) * F]
                scrA = spool.tile([BS, F], f32, tag="scrA")
                scrB = spool.tile([BS, F], f32, tag="scrB")
                th = thpool.tile([BS, 8 * KSB], f32, tag="th")
                m8 = thpool.tile([BS, 8], f32, tag="m8")
                cur, nxt = scrA, scrB
                src = s_head
                for it in range(1, 33):
                    kk = kk_of_iter.get(it)
                    outm = th[:, kk * 8:(kk + 1) * 8] if kk is not None else m8
                    nc.vector.max(out=outm, in_=src)
                    if it < 32:
                        nc.vector.match_replace(
                            out=nxt, in_to_replace=outm, in_values=src, imm_value=NEG
                        )
                        src = nxt
                        cur, nxt = nxt, cur
                # bias terms for the sigmoid: -kth/tau
                nth = thpool.tile([BS, KSB], f32, tag="nth")
                for kk in range(KSB):
                    nc.gpsimd.tensor_scalar_mul(
                        nth[:, kk:kk + 1], th[:, kk * 8 + 7:kk * 8 + 8], -inv_tau
                    )
                nth_all.append(nth)

            # ---- masks, backbone, outputs
            pacc = const.tile([BS, C], f32)
            for h in range(KFC):
                s_head = S_sb[:, h * F:(h + 1) * F]
                nth = nth_all[h]
                for kk in range(KSB):
                    mt = mpool.tile([BS, F], f32)
                    nc.scalar.activation(
                        mt, s_head, AF.Sigmoid, bias=nth[:, kk:kk + 1], scale=inv_tau
                    )
                    nc.sync.dma_start(out=m_d[:, h, kk, :], in_=mt)
                    xm = xmpool.tile([BS, F], f32)
                    nc.gpsimd.tensor_mul(xm, mt, xsb)
                    xmT = xtpool.tile([128, 8, BS], f32)
                    for fc in range(8):
                        pt = psT.tile([128, 128], f32)
                        nc.tensor.transpose(pt, xm[:, fc * 128:(fc + 1) * 128], identity)
                        nc.scalar.copy(xmT[:, fc, :], pt)
                    ph = psH.tile([H, BS], f32)
                    for fc in range(8):
                        nc.tensor.matmul(
                            ph, w1t[:, fc, :], xmT[:, fc, :],
                            start=(fc == 0), stop=(fc == 7),
                        )
                    ht = bbpool.tile([H, BS], f32, tag="ht")
                    nc.scalar.activation(ht, ph, AF.Relu, bias=b1sb[:, 0:1], scale=1.0)
                    pl = psL.tile([C, BS], f32, tag="pl")
                    nc.tensor.matmul(pl, w2sb, ht)
                    lt = bbpool.tile([C, BS], f32, tag="lt")
                    nc.scalar.activation(lt, pl, AF.Identity, bias=b2sb[:, 0:1], scale=1.0)
                    py = psL.tile([BS, C], f32, tag="py")
                    nc.tensor.transpose(py, lt, identity[:C, :C])
                    yt = bbpool.tile([BS, C], f32, tag="yt")
                    nc.scalar.copy(yt, py)
                    nc.sync.dma_start(out=y_d[:, h, kk, :], in_=yt)

                    if kk == KSB - 1:
                        # softmax over classes, accumulated across heads
                        nmx = tiny.tile([BS, 1], f32, tag="nmx")
                        nc.vector.tensor_reduce(
                            nmx, yt, axis=AX.X, op=mybir.AluOpType.max, negate=True
                        )
                        et = bbpool.tile([BS, C], f32, tag="et")
                        ssum = tiny.tile([BS, 1], f32, tag="ssum")
                        nc.scalar.activation(
                            et, yt, AF.Exp, bias=nmx[:, 0:1], scale=1.0, accum_out=ssum
                        )
                        rs = tiny.tile([BS, 1], f32, tag="rs")
                        nc.vector.reciprocal(rs, ssum)
                        pt_ = bbpool.tile([BS, C], f32, tag="pt_")
                        nc.scalar.activation(pt_, et, AF.Copy, bias=0.0, scale=rs[:, 0:1])
                        if h == 0:
                            nc.gpsimd.tensor_copy(pacc, pt_)
                        else:
                            nc.gpsimd.tensor_add(pacc, pacc, pt_)
            nc.gpsimd.tensor_scalar_mul(pacc, pacc, 1.0 / KFC)
            nc.sync.dma_start(out=probs_d[:, :], in_=pacc)

    nc.compile()
    return nc


def _get_nc(inv_tau: float):
    key = round(float(inv_tau), 12)
    if key not in _CACHE:
        _CACHE[key] = _build(inv_tau)
    return _CACHE[key]


def kernel(x, tau, W_s, b_s, W1, b1, W2, b2):
    from concourse.bass_utils import run_bass_kernel_spmd

    x = np.ascontiguousarray(np.asarray(x, np.float32))
    W_s = np.ascontiguousarray(np.asarray(W_s, np.float32))
    b_s = np.ascontiguousarray(np.asarray(b_s, np.float32).reshape(1, KFC * F))
    W1 = np.ascontiguousarray(np.asarray(W1, np.float32))
    b1 = np.ascontiguousarray(np.asarray(b1, np.float32).reshape(H, 1))
    W2 = np.ascontiguousarray(np.asarray(W2, np.float32))
    b2 = np.ascontiguousarray(np.asarray(b2, np.float32).reshape(C, 1))
    inv_tau = 1.0 / float(np.asarray(tau))

    nc = _get_nc(inv_tau)
    in_maps = []
    for c in range(NCORES):
        in_maps.append({
            "x": x[c * BS:(c + 1) * BS],
            "W_s": W_s,
            "b_s": b_s,
            "W1": W1,
            "b1": b1,
            "W2": W2,
            "b2": b2,
        })
    res = run_bass_kernel_spmd(
        nc, in_maps, core_ids=list(range(NCORES)), trace=_TRACE
    )
    global _LAST_RES
    _LAST_RES = res
    probs = np.concatenate([res.results[c]["probs"] for c in range(NCORES)], axis=0)
    Y = np.concatenate([res.results[c]["Y"] for c in range(NCORES)], axis=0)
    M = np.concatenate([res.results[c]["M"] for c in range(NCORES)], axis=0)
    S = np.concatenate(
        [res.results[c]["S"].reshape(BS, KFC, F) for c in range(NCORES)], axis=0
    )
    return probs, Y, M, S


# revision 5
# speedup vs baseline: 136.7507x; 136.7507x over previous
"""BSFSNet (topk_masking) Trainium2 kernel.

Pure data-parallel over 8 NeuronCores: batch B=1024 split into 8 shards of
128 rows; selector/backbone weights replicated.

Per-core pipeline:
  1. S = x @ W_s + b_s            (PE, fp32, PSUM-accumulated over 8 K-chunks)
  2. per (row, head): exact top-k thresholds for k in {32,64,128,256} via
     iterative 8-at-a-time extraction (vector-engine max + match_replace).
     The 8th value of extraction blocks 4/8/16/32 is exactly the k-th
     largest (matches jax.lax.top_k semantics including duplicates).
  3. masks M = sigmoid((S - kth)/tau)  (scalar engine, per-partition bias)
  4. x_masked = x * M  (gpsimd), transposed on PE for the backbone matmuls
  5. h^T = relu(W1^T @ xm^T + b1); logits^T = W2^T @ h^T + b2  (PE + ACT)
  6. Y written back transposed; softmax over classes of the k=256 slice,
     mean over heads -> final_probs.
"""

import sys

try:  # concourse (Bass/Tile) ships with the container, not with this file
    import concourse  # noqa: F401
except ImportError:
    for _p in ("/opt/trn_rl_repo", "/root/.axon_site/_ro/trn_rl_repo"):
        if _p not in sys.path:
            sys.path.insert(0, _p)

import numpy as np

B, F, H, C = 1024, 1024, 128, 100
KFC = 8                      # ranker heads
KLIST = (32, 64, 128, 256)   # hierarchical subset sizes
KSB = len(KLIST)
NCORES = 8
BS = B // NCORES             # batch rows per core
NEG = -3.0e38                # replacement value for extracted maxima

_CACHE = {}
_TRACE = False        # set by test harness to capture an NTFF profile
_LAST_RES = None      # last BassKernelResults (exec_time_ns etc.)


def _build(inv_tau: float):
    from concourse import bacc, mybir
    from concourse import tile
    from concourse.masks import make_identity

    f32 = mybir.dt.float32
    nc = bacc.Bacc("TRN2", target_bir_lowering=False, debug=False)

    x_d = nc.declare_dram_parameter("x", [BS, F], f32, isOutput=False)
    ws_d = nc.declare_dram_parameter("W_s", [F, KFC * F], f32, isOutput=False)
    bs_d = nc.declare_dram_parameter("b_s", [1, KFC * F], f32, isOutput=False)
    w1_d = nc.declare_dram_parameter("W1", [F, H], f32, isOutput=False)
    b1_d = nc.declare_dram_parameter("b1", [H, 1], f32, isOutput=False)
    w2_d = nc.declare_dram_parameter("W2", [H, C], f32, isOutput=False)
    b2_d = nc.declare_dram_parameter("b2", [C, 1], f32, isOutput=False)

    probs_d = nc.declare_dram_parameter("probs", [BS, C], f32, isOutput=True)
    y_d = nc.declare_dram_parameter("Y", [BS, KFC, KSB, C], f32, isOutput=True)
    m_d = nc.declare_dram_parameter("M", [BS, KFC, KSB, F], f32, isOutput=True)
    s_d = nc.declare_dram_parameter("S", [BS, KFC * F], f32, isOutput=True)

    AF = mybir.ActivationFunctionType
    AX = mybir.AxisListType

    with tile.TileContext(nc) as tc:
        with (
            tc.tile_pool(name="const", bufs=1) as const,
            tc.tile_pool(name="wstream", bufs=6) as wpool,
            tc.tile_pool(name="scr", bufs=4) as spool,
            tc.tile_pool(name="th", bufs=8) as thpool,
            tc.tile_pool(name="mask", bufs=4) as mpool,
            tc.tile_pool(name="xm", bufs=3) as xmpool,
            tc.tile_pool(name="xmt", bufs=3) as xtpool,
            tc.tile_pool(name="bb", bufs=4) as bbpool,
            tc.tile_pool(name="tiny", bufs=16) as tiny,
            tc.tile_pool(name="psS", bufs=2, space="PSUM") as psS,
            tc.tile_pool(name="psT", bufs=2, space="PSUM") as psT,
            tc.tile_pool(name="psH", bufs=2, space="PSUM") as psH,
            tc.tile_pool(name="psL", bufs=1, space="PSUM") as psL,
        ):
            identity = const.tile([128, 128], f32)
            make_identity(nc, identity)
            ones1 = const.tile([1, 128], f32)
            nc.gpsimd.memset(ones1, 1.0)

            xsb = const.tile([BS, F], f32)
            nc.sync.dma_start(out=xsb, in_=x_d[:, :])
            bs_sb = const.tile([1, KFC * F], f32)
            nc.sync.dma_start(out=bs_sb, in_=bs_d[:, :])
            w1t = const.tile([128, 8, H], f32)
            for fc in range(8):
                nc.sync.dma_start(out=w1t[:, fc, :], in_=w1_d[fc * 128:(fc + 1) * 128, :])
            w2sb = const.tile([H, C], f32)
            nc.sync.dma_start(out=w2sb, in_=w2_d[:, :])
            b1sb = const.tile([H, 1], f32)
            nc.sync.dma_start(out=b1sb, in_=b1_d[:, :])
            b2sb = const.tile([C, 1], f32)
            nc.sync.dma_start(out=b2sb, in_=b2_d[:, :])

            # x^T tiles for the selector matmul
            xT = const.tile([128, 8, BS], f32)
            for fc in range(8):
                pt = psT.tile([128, 128], f32)
                nc.tensor.transpose(pt, xsb[:, fc * 128:(fc + 1) * 128], identity)
                nc.scalar.copy(xT[:, fc, :], pt)

            # ---- selector: S = x @ W_s + b_s, [BS, 8192] resident in SBUF
            S_sb = const.tile([BS, KFC * F], f32)
            for sc in range(16):
                ps = psS.tile([128, 512], f32)
                for fc in range(8):
                    wst = wpool.tile([128, 512], f32)
                    nc.sync.dma_start(
                        out=wst,
                        in_=ws_d[fc * 128:(fc + 1) * 128, sc * 512:(sc + 1) * 512],
                    )
                    nc.tensor.matmul(ps, xT[:, fc, :], wst, start=(fc == 0), stop=False)
                # += broadcast(b_s) via K=1 matmul of ones^T @ b_s-slice
                nc.tensor.matmul(
                    ps, ones1, bs_sb[0:1, sc * 512:(sc + 1) * 512],
                    start=False, stop=True,
                )
                nc.scalar.copy(S_sb[:, sc * 512:(sc + 1) * 512], ps)
                nc.sync.dma_start(
                    out=s_d[:, sc * 512:(sc + 1) * 512],
                    in_=S_sb[:, sc * 512:(sc + 1) * 512],
                )

            # ---- per head: extract top-256 8-at-a-time; thresholds at 32/64/128/256
            kk_of_iter = {4: 0, 8: 1, 16: 2, 32: 3}
            nth_all = []
            for h in range(KFC):
                s_head = S_sb[:, h * F:(h + 1) * F]
                scrA = spool.tile([BS, F], f32, tag="scrA")
                scrB = spool.tile([BS, F], f32, tag="scrB")
                th = thpool.tile([BS, 8 * KSB], f32, tag="th")
                m8 = thpool.tile([BS, 8], f32, tag="m8")
                cur, nxt = scrA, scrB
                src = s_head
                for it in range(1, 33):
                    kk = kk_of_iter.get(it)
                    outm = th[:, kk * 8:(kk + 1) * 8] if kk is not None else m8
                    nc.vector.max(out=outm, in_=src)
                    if it < 32:
                        nc.vector.match_replace(
                            out=nxt, in_to_replace=outm, in_values=src, imm_value=NEG
                        )
                        src = nxt
                        cur, nxt = nxt, cur
                # bias terms for the sigmoid: -kth/tau
                nth = thpool.tile([BS, KSB], f32, tag="nth")
                for kk in range(KSB):
                    nc.gpsimd.tensor_scalar_mul(
                        nth[:, kk:kk + 1], th[:, kk * 8 + 7:kk * 8 + 8], -inv_tau
                    )
                nth_all.append(nth)

            # ---- masks, backbone, outputs
            pacc = const.tile([BS, C], f32)
            for h in range(KFC):
                s_head = S_sb[:, h * F:(h + 1) * F]
                nth = nth_all[h]
                for kk in range(KSB):
                    mt = mpool.tile([BS, F], f32)
                    nc.scalar.activation(
                        mt, s_head, AF.Sigmoid, bias=nth[:, kk:kk + 1], scale=inv_tau
                    )
                    nc.sync.dma_start(out=m_d[:, h, kk, :], in_=mt)
                    xm = xmpool.tile([BS, F], f32)
                    nc.gpsimd.tensor_mul(xm, mt, xsb)
                    xmT = xtpool.tile([128, 8, BS], f32)
                    for fc in range(8):
                        pt = psT.tile([128, 128], f32)
                        nc.tensor.transpose(pt, xm[:, fc * 128:(fc + 1) * 128], identity)
                        nc.scalar.copy(xmT[:, fc, :], pt)
                    ph = psH.tile([H, BS], f32)
                    for fc in range(8):
                        nc.tensor.matmul(
                            ph, w1t[:, fc, :], xmT[:, fc, :],
                            start=(fc == 0), stop=(fc == 7),
                        )
                    ht = bbpool.tile([H, BS], f32, tag="ht")
                    nc.scalar.activation(ht, ph, AF.Relu, bias=b1sb[:, 0:1], scale=1.0)
                    pl = psL.tile([C, BS], f32, tag="pl")
                    nc.tensor.matmul(pl, w2sb, ht)
                    lt = bbpool.tile([C, BS], f32, tag="lt")
                    nc.scalar.activation(lt, pl, AF.Identity, bias=b2sb[:, 0:1], scale=1.0)
                    py = psL.tile([BS, C], f32, tag="py")
                    nc.tensor.transpose(py, lt, identity[:C, :C])
                    yt = bbpool.tile([BS, C], f32, tag="yt")
                    nc.scalar.copy(yt, py)
                    nc.sync.dma_start(out=y_d[:, h, kk, :], in_=yt)

                    if kk == KSB - 1:
                        # softmax over classes, accumulated across heads
                        nmx = tiny.tile([BS, 1], f32, tag="nmx")
                        nc.vector.tensor_reduce(
                            nmx, yt, axis=AX.X, op=mybir.AluOpType.max, negate=True
                        )
                        et = bbpool.tile([BS, C], f32, tag="et")
                        ssum = tiny.tile([BS, 1], f32, tag="ssum")
                        nc.scalar.activation(
                            et, yt, AF.Exp, bias=nmx[:, 0:1], scale=1.0, accum_out=ssum
                        )
                        rs = tiny.tile([BS, 1], f32, tag="rs")
                        nc.vector.reciprocal(rs, ssum)
                        pt_ = bbpool.tile([BS, C], f32, tag="pt_")
                        nc.scalar.activation(pt_, et, AF.Copy, bias=0.0, scale=rs[:, 0:1])
                        if h == 0:
                            nc.gpsimd.tensor_copy(pacc, pt_)
                        else:
                            nc.gpsimd.tensor_add(pacc, pacc, pt_)
            nc.gpsimd.tensor_scalar_mul(pacc, pacc, 1.0 / KFC)
            nc.sync.dma_start(out=probs_d[:, :], in_=pacc)

    nc.compile()
    return nc


def _get_nc(inv_tau: float):
    key = round(float(inv_tau), 12)
    if key not in _CACHE:
        _CACHE[key] = _build(inv_tau)
    return _CACHE[key]


def kernel(x, tau, W_s, b_s, W1, b1, W2, b2):
    from concourse.bass_utils import run_bass_kernel_spmd

    x = np.ascontiguousarray(np.asarray(x, np.float32))
    W_s = np.ascontiguousarray(np.asarray(W_s, np.float32))
    b_s = np.ascontiguousarray(np.asarray(b_s, np.float32).reshape(1, KFC * F))
    W1 = np.ascontiguousarray(np.asarray(W1, np.float32))
    b1 = np.ascontiguousarray(np.asarray(b1, np.float32).reshape(H, 1))
    W2 = np.ascontiguousarray(np.asarray(W2, np.float32))
    b2 = np.ascontiguousarray(np.asarray(b2, np.float32).reshape(C, 1))
    inv_tau = 1.0 / float(np.asarray(tau))

    nc = _get_nc(inv_tau)
    in_maps = []
    for c in range(NCORES):
        in_maps.append({
            "x": x[c * BS:(c + 1) * BS],
            "W_s": W_s,
            "b_s": b_s,
            "W1": W1,
            "b1": b1,
            "W2": W2,
            "b2": b2,
        })
    res = run_bass_kernel_spmd(
        nc, in_maps, core_ids=list(range(NCORES)), trace=_TRACE
    )
    global _LAST_RES
    _LAST_RES = res
    probs = np.concatenate([res.results[c]["probs"] for c in range(NCORES)], axis=0)
    Y = np.concatenate([res.results[c]["Y"] for c in range(NCORES)], axis=0)
    M = np.concatenate([res.results[c]["M"] for c in range(NCORES)], axis=0)
    S = np.concatenate(
        [res.results[c]["S"].reshape(BS, KFC, F) for c in range(NCORES)], axis=0
    )
    return probs, Y, M, S
